# revision 1
# baseline (speedup 1.0000x reference)
"""BERT input representation kernel for 8 TRN2 NeuronCores.

Math (reference):
    x1  = x @ W_emb + b_emb                      # [B,S,D]
    seg = einsum('bnsd,s->bnd', x1.reshape(B,S/8,8,D), w_seg) + b_seg
    out = (x1.reshape(...) + seg[:,:,None,:]).reshape(B,S,D) + PE(S,D)

Folded form used here (exact algebra):
    out[b,s,:] = (A @ x[b])[s,:] @ W_emb + bias[s,:]
where A = I + blockdiag(ones(8,1) @ w_seg[None,:]) mixes rows within each
8-row segment, and bias[s,:] = PE[s,:] + b_emb*(1 + sum(w_seg)) + b_seg.

Sharding: pure data-parallel over batch; each of 8 cores handles 8 batches
(4096 rows). Per 128-row tile on device:
    mm1 (fp32):  psum_xt[64,128] = x_tile.T @ A^T        (transpose + seg mix)
    cast (ACT):  xt_bf16 = psum_xt                       (PSUM -> SBUF, ->bf16)
    mm2 (bf16):  psum_out[128,1024] = xt_bf16.T @ W_bf16
    add (DVE):   out_sbuf = psum_out + bias_tile         (PSUM -> SBUF)
    DMA store.
"""

import sys

if "/opt/trn_rl_repo" not in sys.path:
    sys.path.insert(0, "/opt/trn_rl_repo")

import ml_dtypes
import numpy as np

import concourse.bacc as bacc
import concourse.mybir as mybir
import concourse.tile as tile
from concourse.bass_utils import run_bass_kernel_spmd

B, S, F, D, SEG = 64, 512, 64, 1024, 8
N_CORES = 8
B_LOC = B // N_CORES          # batches per core
ROWS = B_LOC * S              # 4096 rows per core
TILE_P = 128                  # rows per tile
N_TILES = ROWS // TILE_P      # 32
N_BIAS = S // TILE_P          # 4 distinct bias row-tiles
NH = D // 512                 # matmul free-dim splits

_NC_CACHE = None


def _build_nc():
    nc = bacc.Bacc("TRN2", target_bir_lowering=False, debug=False,
                   num_devices=N_CORES)
    x_d = nc.declare_dram_parameter("x", [ROWS, F], mybir.dt.float32,
                                    isOutput=False)
    at_d = nc.declare_dram_parameter("at", [TILE_P, TILE_P], mybir.dt.float32,
                                     isOutput=False)
    w_d = nc.declare_dram_parameter("w", [F, D], mybir.dt.bfloat16,
                                    isOutput=False)
    b_d = nc.declare_dram_parameter("bias", [S, D], mybir.dt.float32,
                                    isOutput=False)
    out_d = nc.declare_dram_parameter("out", [ROWS, D], mybir.dt.float32,
                                      isOutput=True)

    with tile.TileContext(nc) as tc:
        with (
            tc.tile_pool(name="const", bufs=1) as cpool,
            tc.tile_pool(name="xin", bufs=4) as xpool,
            tc.tile_pool(name="xtb", bufs=4) as xtpool,
            tc.tile_pool(name="outp", bufs=4) as opool,
            tc.tile_pool(name="ps_t", bufs=2, space="PSUM") as pst,
            tc.tile_pool(name="ps_o", bufs=2, space="PSUM") as pso,
        ):
            at_sb = cpool.tile([TILE_P, TILE_P], mybir.dt.float32)
            nc.sync.dma_start(at_sb[:], at_d[:])
            w_sb = cpool.tile([F, D], mybir.dt.bfloat16)
            nc.sync.dma_start(w_sb[:], w_d[:])
            bias_sb = []
            for j in range(N_BIAS):
                bt = cpool.tile([TILE_P, D], mybir.dt.float32, tag=f"bias{j}",
                                name=f"bias{j}")
                nc.sync.dma_start(bt[:], b_d[j * TILE_P:(j + 1) * TILE_P, :])
                bias_sb.append(bt)

            for i in range(N_TILES):
                x_t = xpool.tile([TILE_P, F], mybir.dt.float32, name="x_t")
                nc.sync.dma_start(x_t[:], x_d[i * TILE_P:(i + 1) * TILE_P, :])

                ps_xt = pst.tile([F, TILE_P], mybir.dt.float32, name="ps_xt")
                nc.tensor.matmul(ps_xt[:], x_t[:], at_sb[:],
                                 start=True, stop=True)

                xt_bf = xtpool.tile([F, TILE_P], mybir.dt.bfloat16,
                                    name="xt_bf")
                nc.scalar.copy(xt_bf[:], ps_xt[:])

                ps_out = pso.tile([TILE_P, D], mybir.dt.float32, name="ps_out")
                for h in range(NH):
                    nc.tensor.matmul(ps_out[:, h * 512:(h + 1) * 512],
                                     xt_bf[:], w_sb[:, h * 512:(h + 1) * 512],
                                     start=True, stop=True)

                o_sb = opool.tile([TILE_P, D], mybir.dt.float32, name="o_sb")
                nc.vector.tensor_add(o_sb[:], ps_out[:],
                                     bias_sb[i % N_BIAS][:])
                nc.sync.dma_start(out_d[i * TILE_P:(i + 1) * TILE_P, :],
                                  o_sb[:])
    nc.compile()
    return nc


def _host_constants(W_emb, b_emb, w_seg, b_seg):
    # sinusoidal positional encoding, float32, same formula as the reference
    pos = np.arange(S, dtype=np.float32)[:, None]
    div = np.exp(np.arange(0, D, 2, dtype=np.float32)
                 * (-np.log(10000.0) / D)).astype(np.float32)
    ang = pos * div
    pe = np.zeros((S, D), np.float32)
    pe[:, 0::2] = np.sin(ang)
    pe[:, 1::2] = np.cos(ang)

    bias = (pe + b_emb[None, :] * (np.float32(1.0) + w_seg.sum())
            + b_seg[0]).astype(np.float32)

    blk = np.eye(SEG, dtype=np.float32) + w_seg[:, None] * np.ones(
        (1, SEG), np.float32)
    at = np.kron(np.eye(TILE_P // SEG, dtype=np.float32), blk).astype(
        np.float32)

    wb = W_emb.astype(ml_dtypes.bfloat16)
    return at, wb, bias


def kernel(x, W_emb, b_emb, w_seg, b_seg):
    x = np.ascontiguousarray(np.asarray(x, dtype=np.float32))
    W_emb = np.asarray(W_emb, dtype=np.float32)
    b_emb = np.asarray(b_emb, dtype=np.float32)
    w_seg = np.asarray(w_seg, dtype=np.float32)
    b_seg = np.asarray(b_seg, dtype=np.float32)

    at, wb, bias = _host_constants(W_emb, b_emb, w_seg, b_seg)

    in_maps = []
    for c in range(N_CORES):
        xs = np.ascontiguousarray(
            x[c * B_LOC:(c + 1) * B_LOC].reshape(ROWS, F))
        in_maps.append({"x": xs, "at": at, "w": wb, "bias": bias})

    global _NC_CACHE
    if _NC_CACHE is None:
        _NC_CACHE = _build_nc()

    res = run_bass_kernel_spmd(_NC_CACHE, in_maps,
                               core_ids=list(range(N_CORES)))
    out = np.concatenate(
        [np.asarray(res.results[c]["out"]).reshape(B_LOC, S, D)
         for c in range(N_CORES)], axis=0)
    return out


# revision 3
# speedup vs baseline: 1.2531x; 1.2531x over previous
"""BERT input representation kernel for 8 TRN2 NeuronCores.

Math (reference):
    x1  = x @ W_emb + b_emb                      # [B,S,D]
    seg = einsum('bnsd,s->bnd', x1.reshape(B,S/8,8,D), w_seg) + b_seg
    out = (x1.reshape(...) + seg[:,:,None,:]).reshape(B,S,D) + PE(S,D)

Folded form used here (exact algebra):
    out[b,s,:] = (A @ x[b])[s,:] @ W_emb + bias[s,:]
where A = I + blockdiag(ones(8,1) @ w_seg[None,:]) mixes rows within each
8-row segment, and bias[s,:] = PE[s,:] + b_emb*(1 + sum(w_seg)) + b_seg.

Sharding: pure data-parallel over batch; each of 8 cores handles 8 batches
(4096 rows = 32 row-tiles of 128). Per core:
    one SWDGE cast-DMA loads all of x (f32 -> bf16) into SBUF resident
    per 128-row tile i:
      mm1 (bf16): psum_xt[64,128] = x_tile.T @ A^T     (transpose + seg mix)
      cast (ACT): xt_bf16 = psum_xt                    (PSUM -> SBUF)
      mm2 (bf16): psum_out[128,1024] = xt_bf16.T @ W_bf16
      add (DVE):  out_sbuf = psum_out + bias_tile      (PSUM -> SBUF, f32)
      DMA store (alternating between the two HWDGE rings).
"""

import sys

if "/opt/trn_rl_repo" not in sys.path:
    sys.path.insert(0, "/opt/trn_rl_repo")

import ml_dtypes
import numpy as np

import concourse.bacc as bacc
import concourse.mybir as mybir
import concourse.tile as tile
from concourse.bass_utils import run_bass_kernel_spmd

B, S, F, D, SEG = 64, 512, 64, 1024, 8
N_CORES = 8
B_LOC = B // N_CORES          # batches per core
ROWS = B_LOC * S              # 4096 rows per core
TILE_P = 128                  # rows per tile
N_TILES = ROWS // TILE_P      # 32
N_BIAS = S // TILE_P          # 4 distinct bias row-tiles
NH = D // 512                 # matmul free-dim splits

_NC_CACHE = None


def _build_nc():
    nc = bacc.Bacc("TRN2", target_bir_lowering=False, debug=False,
                   num_devices=N_CORES)
    # x pre-rearranged on host (layout only): xr[p, i*F:(i+1)*F] = x[i*128+p]
    x_d = nc.declare_dram_parameter("x", [TILE_P, N_TILES * F],
                                    mybir.dt.float32, isOutput=False)
    at_d = nc.declare_dram_parameter("at", [TILE_P, TILE_P],
                                     mybir.dt.bfloat16, isOutput=False)
    w_d = nc.declare_dram_parameter("w", [F, D], mybir.dt.bfloat16,
                                    isOutput=False)
    # bias rearranged: [128, 4*D], column block j = bias rows j*128..j*128+127
    b_d = nc.declare_dram_parameter("bias", [TILE_P, N_BIAS * D],
                                    mybir.dt.float32, isOutput=False)
    out_d = nc.declare_dram_parameter("out", [ROWS, D], mybir.dt.float32,
                                      isOutput=True)

    with tile.TileContext(nc) as tc:
        with (
            tc.tile_pool(name="const", bufs=1) as cpool,
            tc.tile_pool(name="xtb", bufs=4) as xtpool,
            tc.tile_pool(name="outp", bufs=4) as opool,
            tc.tile_pool(name="ps_t", bufs=2, space="PSUM") as pst,
            tc.tile_pool(name="ps_o", bufs=3, space="PSUM") as pso,
        ):
            # resident inputs
            x_sb = cpool.tile([TILE_P, N_TILES * F], mybir.dt.bfloat16)
            nc.gpsimd.dma_start(x_sb[:], x_d[:])       # SWDGE f32->bf16 cast
            at_sb = cpool.tile([TILE_P, TILE_P], mybir.dt.bfloat16)
            nc.sync.dma_start(at_sb[:], at_d[:])
            w_sb = cpool.tile([F, D], mybir.dt.bfloat16)
            nc.sync.dma_start(w_sb[:], w_d[:])
            bias_sb = cpool.tile([TILE_P, N_BIAS * D], mybir.dt.float32)
            nc.sync.dma_start(bias_sb[:], b_d[:])

            for i in range(N_TILES):
                ps_xt = pst.tile([F, TILE_P], mybir.dt.float32, name="ps_xt")
                nc.tensor.matmul(ps_xt[:], x_sb[:, i * F:(i + 1) * F],
                                 at_sb[:], start=True, stop=True)

                xt_bf = xtpool.tile([F, TILE_P], mybir.dt.bfloat16,
                                    name="xt_bf")
                nc.scalar.copy(xt_bf[:], ps_xt[:])

                ps_out = pso.tile([TILE_P, D], mybir.dt.float32, name="ps_out")
                for h in range(NH):
                    nc.tensor.matmul(ps_out[:, h * 512:(h + 1) * 512],
                                     xt_bf[:], w_sb[:, h * 512:(h + 1) * 512],
                                     start=True, stop=True)

                o_sb = opool.tile([TILE_P, D], mybir.dt.float32, name="o_sb")
                j = i % N_BIAS
                nc.vector.tensor_add(o_sb[:], ps_out[:],
                                     bias_sb[:, j * D:(j + 1) * D])
                store_eng = nc.sync if i % 2 == 0 else nc.scalar
                store_eng.dma_start(out_d[i * TILE_P:(i + 1) * TILE_P, :],
                                    o_sb[:])
    nc.compile()
    return nc


def _host_constants(W_emb, b_emb, w_seg, b_seg):
    # sinusoidal positional encoding, float32, same formula as the reference
    pos = np.arange(S, dtype=np.float32)[:, None]
    div = np.exp(np.arange(0, D, 2, dtype=np.float32)
                 * (-np.log(10000.0) / D)).astype(np.float32)
    ang = pos * div
    pe = np.zeros((S, D), np.float32)
    pe[:, 0::2] = np.sin(ang)
    pe[:, 1::2] = np.cos(ang)

    bias = (pe + b_emb[None, :] * (np.float32(1.0) + w_seg.sum())
            + b_seg[0]).astype(np.float32)
    # rearrange to [128, 4*D]: column block j holds bias rows j*128..j*128+127
    bias_r = np.ascontiguousarray(
        bias.reshape(N_BIAS, TILE_P, D).transpose(1, 0, 2).reshape(
            TILE_P, N_BIAS * D))

    blk = np.eye(SEG, dtype=np.float32) + w_seg[:, None] * np.ones(
        (1, SEG), np.float32)
    at = np.kron(np.eye(TILE_P // SEG, dtype=np.float32), blk).astype(
        ml_dtypes.bfloat16)

    wb = W_emb.astype(ml_dtypes.bfloat16)
    return at, wb, bias_r


def _prepare_in_maps(x, W_emb, b_emb, w_seg, b_seg):
    x = np.ascontiguousarray(np.asarray(x, dtype=np.float32))
    W_emb = np.asarray(W_emb, dtype=np.float32)
    b_emb = np.asarray(b_emb, dtype=np.float32)
    w_seg = np.asarray(w_seg, dtype=np.float32)
    b_seg = np.asarray(b_seg, dtype=np.float32)

    at, wb, bias_r = _host_constants(W_emb, b_emb, w_seg, b_seg)

    in_maps = []
    for c in range(N_CORES):
        xs = x[c * B_LOC:(c + 1) * B_LOC].reshape(ROWS, F)
        # layout-only rearrange: [32 tiles, 128 rows, F] -> [128, 32*F]
        xr = np.ascontiguousarray(
            xs.reshape(N_TILES, TILE_P, F).transpose(1, 0, 2).reshape(
                TILE_P, N_TILES * F))
        in_maps.append({"x": xr, "at": at, "w": wb, "bias": bias_r})
    return in_maps


def kernel(x, W_emb, b_emb, w_seg, b_seg):
    in_maps = _prepare_in_maps(x, W_emb, b_emb, w_seg, b_seg)

    global _NC_CACHE
    if _NC_CACHE is None:
        _NC_CACHE = _build_nc()

    res = run_bass_kernel_spmd(_NC_CACHE, in_maps,
                               core_ids=list(range(N_CORES)))
    out = np.concatenate(
        [np.asarray(res.results[c]["out"]).reshape(B_LOC, S, D)
         for c in range(N_CORES)], axis=0)
    return out


# revision 6
# speedup vs baseline: 1.2968x; 1.0349x over previous
"""BERT input representation kernel for 8 TRN2 NeuronCores.

Math (reference):
    x1  = x @ W_emb + b_emb                      # [B,S,D]
    seg = einsum('bnsd,s->bnd', x1.reshape(B,S/8,8,D), w_seg) + b_seg
    out = (x1.reshape(...) + seg[:,:,None,:]).reshape(B,S,D) + PE(S,D)

Folded form used here (exact algebra):
    out[b,s,:] = (A @ x[b])[s,:] @ W_emb + bias[s,:]
where A = I + blockdiag(ones(8,1) @ w_seg[None,:]) mixes rows within each
8-row segment, and bias[s,:] = PE[s,:] + b_emb*(1 + sum(w_seg)) + b_seg.

Sharding: pure data-parallel over batch; each of 8 cores handles 8 batches
(4096 rows = 32 row-tiles of 128 rows). Device schedule per core:
  - load all of x [128, 32*64] f32 (host did a layout-only rearrange so
    partition p holds row p of every tile), cast to bf16 on DVE
  - phase 1 (also serves as PE warm-up burst): 16 matmuls, each computing
    x~^T for a PAIR of row-tiles:  psum[128, 128] = x2.T @ A^T where x2
    stacks two tiles' 64 features; ACT copies psum -> resident xt bf16
  - phase 2, per pair of row-tiles (16 groups):
      PE: preload bias (high 512 cols) into PSUM via identity matmul,
          then out_psum = xt.T @ W (low half start=True, high half
          accumulates onto the preloaded bias)
      DVE: out_sbuf[low 512]  = out_psum[low]  + bias   (tensor_add)
      ACT: out_sbuf[high 512] = out_psum[high]          (plain copy)
      one 1 MiB store per group (two row-tiles) on the sync HWDGE ring
"""

import sys

if "/opt/trn_rl_repo" not in sys.path:
    sys.path.insert(0, "/opt/trn_rl_repo")

import ml_dtypes
import numpy as np

import concourse.bacc as bacc
import concourse.mybir as mybir
import concourse.tile as tile
from concourse.bass_utils import run_bass_kernel_spmd

B, S, F, D, SEG = 64, 512, 64, 1024, 8
N_CORES = 8
B_LOC = B // N_CORES          # batches per core
ROWS = B_LOC * S              # 4096 rows per core
TILE_P = 128                  # rows per tile
N_TILES = ROWS // TILE_P      # 32
N_PAIR = N_TILES // 2         # 16 tile-pairs
N_BIAS = S // TILE_P          # 4 distinct bias row-tiles
HD = D // 2                   # 512

_NC_CACHE = None


def _build_nc():
    nc = bacc.Bacc("TRN2", target_bir_lowering=False, debug=False,
                   num_devices=N_CORES)
    # x pre-rearranged on host (layout only): xr[p, i*F:(i+1)*F] = x[i*128+p]
    x_d = nc.declare_dram_parameter("x", [TILE_P, N_TILES * F],
                                    mybir.dt.float32, isOutput=False)
    at_d = nc.declare_dram_parameter("at", [TILE_P, TILE_P],
                                     mybir.dt.bfloat16, isOutput=False)
    id_d = nc.declare_dram_parameter("ident", [TILE_P, TILE_P],
                                     mybir.dt.bfloat16, isOutput=False)
    # W stacked twice on host: partitions 0-63 and 64-127 both hold W,
    # so mains with lhsT at base_partition 64 have a matching-base rhs.
    w_d = nc.declare_dram_parameter("w", [2 * F, D], mybir.dt.bfloat16,
                                    isOutput=False)
    # bias rearranged: [128, 4*D], column block j = bias rows j*128..j*128+127
    b_d = nc.declare_dram_parameter("bias", [TILE_P, N_BIAS * D],
                                    mybir.dt.bfloat16, isOutput=False)
    out_d = nc.declare_dram_parameter("out", [ROWS, D], mybir.dt.float32,
                                      isOutput=True)

    with tile.TileContext(nc) as tc:
        with (
            tc.tile_pool(name="const", bufs=1) as cpool,
            tc.tile_pool(name="outp", bufs=3) as opool,
            tc.tile_pool(name="ps_t", bufs=2, space="PSUM") as pst,
            tc.tile_pool(name="ps_o", bufs=3, space="PSUM") as pso,
        ):
            x32 = cpool.tile([TILE_P, N_TILES * F], mybir.dt.float32)
            nc.sync.dma_start(x32[:], x_d[:])
            bias_sb = cpool.tile([TILE_P, N_BIAS * D], mybir.dt.bfloat16)
            nc.scalar.dma_start(bias_sb[:], b_d[:])
            w_sb = cpool.tile([2 * F, D], mybir.dt.bfloat16)
            nc.scalar.dma_start(w_sb[:], w_d[:])
            at_sb = cpool.tile([TILE_P, TILE_P], mybir.dt.bfloat16)
            nc.scalar.dma_start(at_sb[:], at_d[:])
            i_sb = cpool.tile([TILE_P, TILE_P], mybir.dt.bfloat16)
            nc.scalar.dma_start(i_sb[:], id_d[:])

            x_bf = cpool.tile([TILE_P, N_TILES * F], mybir.dt.bfloat16)
            nc.vector.tensor_copy(x_bf[:], x32[:])

            # resident x~^T (bf16): xt_sb[64u+f, 128*pr+n] = x~[2pr+u, n, f]
            xt_sb = cpool.tile([TILE_P, N_PAIR * TILE_P], mybir.dt.bfloat16)

            # phase 1: transpose + segment-mix, 4 pairs per PSUM bank
            for b4 in range(N_PAIR // 4):
                ps_x = pst.tile([TILE_P, 512], mybir.dt.float32, name="ps_x")
                for k in range(4):
                    pr = 4 * b4 + k
                    nc.tensor.matmul(ps_x[:, 128 * k:128 * (k + 1)],
                                     x_bf[:, 128 * pr:128 * (pr + 1)],
                                     at_sb[:], start=True, stop=True)
                nc.scalar.copy(xt_sb[:, 512 * b4:512 * (b4 + 1)], ps_x[:])

            # phase 2: matmul + bias + store, two row-tiles per group
            for j in range(N_PAIR):
                o_sb = opool.tile([TILE_P, 2 * D], mybir.dt.float32,
                                  name="o_sb")
                for u in range(2):
                    i = 2 * j + u
                    jb = i % N_BIAS
                    lhs = xt_sb[64 * u:64 * (u + 1),
                                128 * j:128 * (j + 1)]
                    ps = pso.tile([TILE_P, D], mybir.dt.float32, name="ps")
                    # preload bias into the high half of PSUM
                    nc.tensor.matmul(ps[:, HD:D], i_sb[:],
                                     bias_sb[:, jb * D + HD:(jb + 1) * D],
                                     start=True, stop=False)
                    wsl = w_sb[64 * u:64 * u + F, :]
                    nc.tensor.matmul(ps[:, HD:D], lhs, wsl[:, HD:D],
                                     start=False, stop=True)
                    nc.tensor.matmul(ps[:, 0:HD], lhs, wsl[:, 0:HD],
                                     start=True, stop=True)
                    nc.vector.tensor_add(o_sb[:, u * D:u * D + HD],
                                         ps[:, 0:HD],
                                         bias_sb[:, jb * D:jb * D + HD])
                    nc.scalar.copy(o_sb[:, u * D + HD:(u + 1) * D],
                                   ps[:, HD:D])
                dram = out_d[j * 256:(j + 1) * 256, :].rearrange(
                    "(a p) d -> p a d", a=2, p=TILE_P)
                nc.sync.dma_start(dram, o_sb[:].rearrange(
                    "p (a d) -> p a d", a=2))
    nc.compile()
    return nc


def _host_constants(W_emb, b_emb, w_seg, b_seg):
    # sinusoidal positional encoding, float32, same formula as the reference
    pos = np.arange(S, dtype=np.float32)[:, None]
    div = np.exp(np.arange(0, D, 2, dtype=np.float32)
                 * (-np.log(10000.0) / D)).astype(np.float32)
    ang = pos * div
    pe = np.zeros((S, D), np.float32)
    pe[:, 0::2] = np.sin(ang)
    pe[:, 1::2] = np.cos(ang)

    bias = (pe + b_emb[None, :] * (np.float32(1.0) + w_seg.sum())
            + b_seg[0]).astype(np.float32)
    # rearrange to [128, 4*D]: column block j holds bias rows j*128..j*128+127
    bias_r = np.ascontiguousarray(
        bias.reshape(N_BIAS, TILE_P, D).transpose(1, 0, 2).reshape(
            TILE_P, N_BIAS * D)).astype(ml_dtypes.bfloat16)

    blk = np.eye(SEG, dtype=np.float32) + w_seg[:, None] * np.ones(
        (1, SEG), np.float32)
    at = np.kron(np.eye(TILE_P // SEG, dtype=np.float32), blk).astype(
        ml_dtypes.bfloat16)

    ident = np.eye(TILE_P, dtype=np.float32).astype(ml_dtypes.bfloat16)
    wb = np.ascontiguousarray(
        np.vstack([W_emb, W_emb])).astype(ml_dtypes.bfloat16)
    return at, ident, wb, bias_r


def _prepare_in_maps(x, W_emb, b_emb, w_seg, b_seg):
    x = np.ascontiguousarray(np.asarray(x, dtype=np.float32))
    W_emb = np.asarray(W_emb, dtype=np.float32)
    b_emb = np.asarray(b_emb, dtype=np.float32)
    w_seg = np.asarray(w_seg, dtype=np.float32)
    b_seg = np.asarray(b_seg, dtype=np.float32)

    at, ident, wb, bias_r = _host_constants(W_emb, b_emb, w_seg, b_seg)

    in_maps = []
    for c in range(N_CORES):
        xs = x[c * B_LOC:(c + 1) * B_LOC].reshape(ROWS, F)
        # layout-only rearrange: [32 tiles, 128 rows, F] -> [128, 32*F]
        xr = np.ascontiguousarray(
            xs.reshape(N_TILES, TILE_P, F).transpose(1, 0, 2).reshape(
                TILE_P, N_TILES * F))
        in_maps.append({"x": xr, "at": at, "ident": ident, "w": wb,
                        "bias": bias_r})
    return in_maps


def kernel(x, W_emb, b_emb, w_seg, b_seg):
    in_maps = _prepare_in_maps(x, W_emb, b_emb, w_seg, b_seg)

    global _NC_CACHE
    if _NC_CACHE is None:
        _NC_CACHE = _build_nc()

    res = run_bass_kernel_spmd(_NC_CACHE, in_maps,
                               core_ids=list(range(N_CORES)))
    out = np.concatenate(
        [np.asarray(res.results[c]["out"]).reshape(B_LOC, S, D)
         for c in range(N_CORES)], axis=0)
    return out


# revision 8
# speedup vs baseline: 1.3240x; 1.0209x over previous
"""BERT input representation kernel for 8 TRN2 NeuronCores.

Math (reference):
    x1  = x @ W_emb + b_emb                      # [B,S,D]
    seg = einsum('bnsd,s->bnd', x1.reshape(B,S/8,8,D), w_seg) + b_seg
    out = (x1.reshape(...) + seg[:,:,None,:]).reshape(B,S,D) + PE(S,D)

Folded form used here (exact algebra):
    out[b,s,:] = (A @ x[b])[s,:] @ W_emb + bias[s,:]
where A = I + blockdiag(ones(8,1) @ w_seg[None,:]) mixes rows within each
8-row segment, and bias[s,:] = PE[s,:] + b_emb*(1 + sum(w_seg)) + b_seg.

Sharding: pure data-parallel over batch; each of 8 cores handles 8 batches
(4096 rows = 32 row-tiles of 128 rows). Device schedule per core:
  - load all of x [128, 32*64] f32 (host did a layout-only rearrange so
    partition p holds row p of every tile), cast to bf16 on DVE
  - phase 1 (also serves as PE warm-up burst): 16 matmuls, each computing
    x~^T for a PAIR of row-tiles:  psum[128, 128] = x2.T @ A^T where x2
    stacks two tiles' 64 features; ACT copies psum -> resident xt bf16
  - phase 2, per pair of row-tiles (16 groups):
      PE: preload bias (high 512 cols) into PSUM via identity matmul,
          then out_psum = xt.T @ W (low half start=True, high half
          accumulates onto the preloaded bias)
      DVE: out_sbuf[low 512]  = out_psum[low]  + bias   (tensor_add)
      ACT: out_sbuf[high 512] = out_psum[high]          (plain copy)
      one 1 MiB store per group (two row-tiles) on the sync HWDGE ring
"""

import sys

if "/opt/trn_rl_repo" not in sys.path:
    sys.path.insert(0, "/opt/trn_rl_repo")

import ml_dtypes
import numpy as np

import concourse.bacc as bacc
import concourse.mybir as mybir
import concourse.tile as tile
from concourse.bass_utils import run_bass_kernel_spmd

B, S, F, D, SEG = 64, 512, 64, 1024, 8
N_CORES = 8
B_LOC = B // N_CORES          # batches per core
ROWS = B_LOC * S              # 4096 rows per core
TILE_P = 128                  # rows per tile
N_TILES = ROWS // TILE_P      # 32
N_PAIR = N_TILES // 2         # 16 tile-pairs
N_BIAS = S // TILE_P          # 4 distinct bias row-tiles
HD = D // 2                   # 512

_NC_CACHE = None


def _build_nc():
    nc = bacc.Bacc("TRN2", target_bir_lowering=False, debug=False,
                   num_devices=N_CORES)
    # x pre-rearranged on host (layout only): xr[p, i*F:(i+1)*F] = x[i*128+p]
    x_d = nc.declare_dram_parameter("x", [TILE_P, N_TILES * F],
                                    mybir.dt.float32, isOutput=False)
    at_d = nc.declare_dram_parameter("at", [TILE_P, TILE_P],
                                     mybir.dt.bfloat16, isOutput=False)
    id_d = nc.declare_dram_parameter("ident", [TILE_P, TILE_P],
                                     mybir.dt.bfloat16, isOutput=False)
    # W stacked twice on host: partitions 0-63 and 64-127 both hold W,
    # so mains with lhsT at base_partition 64 have a matching-base rhs.
    w_d = nc.declare_dram_parameter("w", [2 * F, D], mybir.dt.bfloat16,
                                    isOutput=False)
    # bias rearranged: [128, 4*D], column block j = bias rows j*128..j*128+127
    b_d = nc.declare_dram_parameter("bias", [TILE_P, N_BIAS * D],
                                    mybir.dt.bfloat16, isOutput=False)
    out_d = nc.declare_dram_parameter("out", [ROWS, D], mybir.dt.float32,
                                      isOutput=True)

    with tile.TileContext(nc) as tc:
        with (
            tc.tile_pool(name="const", bufs=1) as cpool,
            tc.tile_pool(name="xin", bufs=2) as xpool,
            tc.tile_pool(name="xbf", bufs=2) as xbpool,
            tc.tile_pool(name="outp", bufs=3) as opool,
            tc.tile_pool(name="ps_t", bufs=2, space="PSUM") as pst,
            tc.tile_pool(name="ps_o", bufs=3, space="PSUM") as pso,
        ):
            bias_sb = cpool.tile([TILE_P, N_BIAS * D], mybir.dt.bfloat16)
            nc.scalar.dma_start(bias_sb[:], b_d[:])
            w_sb = cpool.tile([2 * F, D], mybir.dt.bfloat16)
            nc.scalar.dma_start(w_sb[:], w_d[:])
            at_sb = cpool.tile([TILE_P, TILE_P], mybir.dt.bfloat16)
            nc.scalar.dma_start(at_sb[:], at_d[:])
            i_sb = cpool.tile([TILE_P, TILE_P], mybir.dt.bfloat16)
            nc.scalar.dma_start(i_sb[:], id_d[:])

            # resident x~^T (bf16): xt_sb[64u+f, 128*pr+n] = x~[2pr+u, n, f]
            xt_sb = cpool.tile([TILE_P, N_PAIR * TILE_P], mybir.dt.bfloat16)

            # 4 waves; each wave loads an x chunk, builds x~^T for 4 pairs
            # (phase 1), then runs matmul+bias+store for those 4 groups
            # (phase 2) — so the first output store launches early and the
            # PE stream stays dense.
            for b4 in range(N_PAIR // 4):
                xc = xpool.tile([TILE_P, 512], mybir.dt.float32, name="xc")
                nc.sync.dma_start(xc[:], x_d[:, 512 * b4:512 * (b4 + 1)])
                xcb = xbpool.tile([TILE_P, 512], mybir.dt.bfloat16,
                                  name="xcb")
                nc.vector.tensor_copy(xcb[:], xc[:])

                ps_x = pst.tile([TILE_P, 512], mybir.dt.float32, name="ps_x")
                for k in range(4):
                    nc.tensor.matmul(ps_x[:, 128 * k:128 * (k + 1)],
                                     xcb[:, 128 * k:128 * (k + 1)],
                                     at_sb[:], start=True, stop=True)
                nc.scalar.copy(xt_sb[:, 512 * b4:512 * (b4 + 1)], ps_x[:])

                for j in range(4 * b4, 4 * b4 + 4):
                    o_sb = opool.tile([TILE_P, 2 * D], mybir.dt.float32,
                                      name="o_sb")
                    for u in range(2):
                        i = 2 * j + u
                        jb = i % N_BIAS
                        lhs = xt_sb[64 * u:64 * (u + 1),
                                    128 * j:128 * (j + 1)]
                        ps = pso.tile([TILE_P, D], mybir.dt.float32,
                                      name="ps")
                        # preload bias into the high half of PSUM
                        nc.tensor.matmul(ps[:, HD:D], i_sb[:],
                                         bias_sb[:, jb * D + HD:(jb + 1) * D],
                                         start=True, stop=False)
                        wsl = w_sb[64 * u:64 * u + F, :]
                        nc.tensor.matmul(ps[:, HD:D], lhs, wsl[:, HD:D],
                                         start=False, stop=True)
                        nc.tensor.matmul(ps[:, 0:HD], lhs, wsl[:, 0:HD],
                                         start=True, stop=True)
                        nc.vector.tensor_add(o_sb[:, u * D:u * D + HD],
                                             ps[:, 0:HD],
                                             bias_sb[:, jb * D:jb * D + HD])
                        nc.scalar.copy(o_sb[:, u * D + HD:(u + 1) * D],
                                       ps[:, HD:D])
                    dram = out_d[j * 256:(j + 1) * 256, :].rearrange(
                        "(a p) d -> p a d", a=2, p=TILE_P)
                    nc.sync.dma_start(dram, o_sb[:].rearrange(
                        "p (a d) -> p a d", a=2))
    nc.compile()
    return nc


def _host_constants(W_emb, b_emb, w_seg, b_seg):
    # sinusoidal positional encoding, float32, same formula as the reference
    pos = np.arange(S, dtype=np.float32)[:, None]
    div = np.exp(np.arange(0, D, 2, dtype=np.float32)
                 * (-np.log(10000.0) / D)).astype(np.float32)
    ang = pos * div
    pe = np.zeros((S, D), np.float32)
    pe[:, 0::2] = np.sin(ang)
    pe[:, 1::2] = np.cos(ang)

    bias = (pe + b_emb[None, :] * (np.float32(1.0) + w_seg.sum())
            + b_seg[0]).astype(np.float32)
    # rearrange to [128, 4*D]: column block j holds bias rows j*128..j*128+127
    bias_r = np.ascontiguousarray(
        bias.reshape(N_BIAS, TILE_P, D).transpose(1, 0, 2).reshape(
            TILE_P, N_BIAS * D)).astype(ml_dtypes.bfloat16)

    blk = np.eye(SEG, dtype=np.float32) + w_seg[:, None] * np.ones(
        (1, SEG), np.float32)
    at = np.kron(np.eye(TILE_P // SEG, dtype=np.float32), blk).astype(
        ml_dtypes.bfloat16)

    ident = np.eye(TILE_P, dtype=np.float32).astype(ml_dtypes.bfloat16)
    wb = np.ascontiguousarray(
        np.vstack([W_emb, W_emb])).astype(ml_dtypes.bfloat16)
    return at, ident, wb, bias_r


def _prepare_in_maps(x, W_emb, b_emb, w_seg, b_seg):
    x = np.ascontiguousarray(np.asarray(x, dtype=np.float32))
    W_emb = np.asarray(W_emb, dtype=np.float32)
    b_emb = np.asarray(b_emb, dtype=np.float32)
    w_seg = np.asarray(w_seg, dtype=np.float32)
    b_seg = np.asarray(b_seg, dtype=np.float32)

    at, ident, wb, bias_r = _host_constants(W_emb, b_emb, w_seg, b_seg)

    in_maps = []
    for c in range(N_CORES):
        xs = x[c * B_LOC:(c + 1) * B_LOC].reshape(ROWS, F)
        # layout-only rearrange: [32 tiles, 128 rows, F] -> [128, 32*F]
        xr = np.ascontiguousarray(
            xs.reshape(N_TILES, TILE_P, F).transpose(1, 0, 2).reshape(
                TILE_P, N_TILES * F))
        in_maps.append({"x": xr, "at": at, "ident": ident, "w": wb,
                        "bias": bias_r})
    return in_maps


def kernel(x, W_emb, b_emb, w_seg, b_seg):
    in_maps = _prepare_in_maps(x, W_emb, b_emb, w_seg, b_seg)

    global _NC_CACHE
    if _NC_CACHE is None:
        _NC_CACHE = _build_nc()

    res = run_bass_kernel_spmd(_NC_CACHE, in_maps,
                               core_ids=list(range(N_CORES)))
    out = np.concatenate(
        [np.asarray(res.results[c]["out"]).reshape(B_LOC, S, D)
         for c in range(N_CORES)], axis=0)
    return out
